# revision 36
# baseline (speedup 1.0000x reference)
"""Trainium2 Bass kernel for nn_MixMLP (moe_routing) — fp8 DoubleRow, v4.

Strategy:
  - Output is binary: y_hard + y_soft - stop_grad(y_soft) == y_hard numerically,
    so each edge decision is  (logit0 - logit1) + (gum0 - gum1) >= 0.
  - Only the DIFFERENCE of adjacent final-layer columns matters:
        d = h3 @ wd,  wd = w3[:, 0::2] - w3[:, 1::2]   (1024 x 32640)
    decision = (d + bdd - gd) >= 0,  gd = gum1 - gum0, bdd = b3[0::2]-b3[1::2].
  - Rows are routed to one expert by mask = x[:,0] > 0. Host sorts rows so
    row-chunks of 128 are single-expert; 8 cores = 2 row-groups (one per
    expert) x 4 column-quarters of wd.
  - Device computes ONLY d (scaled). MLP layers 1-2 in bf16; h2 emitted as
    fp8 so the h3 layer and the big layer both run DoubleRow fp8
    (256-contraction, 2x rate). Output d in fp8 (x SCALE_OUT). gumbel never
    touches the device.
  - Perf structure: one coalesced bf16 input blob + one fp8 w2 + 8x1MB wd
    stream, all on the sync HWDGE ring in FIFO order (small tensors first);
    2-bank PSUM tiles evacuated by single [128,1024] copies alternating
    ACT/DVE; output DMAs (4 x 0.5MB fp8) on the scalar HWDGE ring.
  - Host: dec = (t >= 0), t = d + bdd - gd; near-ties |t| < 7e-3+0.05|d|
    recomputed exactly in float64, then scattered into the symmetric
    adjacency. Result is exact regardless of device matmul precision.
"""

import os
import numpy as np
import ml_dtypes

import concourse.bass as bass
import concourse.mybir as mybir
import concourse.tile as tile
from concourse import bacc
from concourse.bass_utils import run_bass_kernel_spmd

B = 512
COND = 64
N_NODES = 256
E = 32640  # upper-tri edges
NCORES = 8
QCOLS = E // 4  # 8160 columns of wd per core
QP = 8192  # padded to multiple of 1024
ARCH = [256, 512, 1024]

ALPHA = 512.0  # wd scale into fp8
BETA = 32.0  # h3 scale into fp8
GAMMA = 32.0  # h2 scale into fp8
W2S = 16.0  # w2 scale into fp8
SCALE = ALPHA * BETA  # big-layer psum holds SCALE*d
SCALE_OUT = 512.0  # output tensor holds SCALE_OUT*d

F32 = mybir.dt.float32
BF16 = mybir.dt.bfloat16
FP8 = mybir.dt.float8e4
NP_FP8 = ml_dtypes.float8_e4m3
NP_BF16 = ml_dtypes.bfloat16

# |t| < TOL_ABS + TOL_REL*|d| edges are recomputed exactly on host
TOL_ABS = 7.0e-3
TOL_REL = 0.05  # covers fp8 output quantization of d

# input blob column layout (bf16, 128 partitions), R = nslots*128:
#   [0:R)           xT   (64 partitions used)
#   [R:R+256)       w0   (64 partitions used)
#   [R+256:R+270)   packed biases: b0 x2 | GAMMA*b1 x4 | BETA*b2 x8
#   [R+270:R+1294)  w1   as k-major [k=0 512 cols | k=1 512 cols]
# it is DMA'd in two pieces: [0:R+270) first (unblocks h1), then w1
def blob_cols(R):
    return R + 1294

_program_cache = {}
last_results = None  # BassKernelResults of the most recent device run


def build_program(nslots: int):
    """One SPMD program: R = nslots*128 rows, one expert, one wd quarter."""
    R = nslots * 128
    DR = mybir.MatmulPerfMode.DoubleRow
    nc = bacc.Bacc(None, target_bir_lowering=False)

    BC = blob_cols(R)
    blob = nc.dram_tensor("blob", [128, BC], BF16, kind="ExternalInput")
    w2 = nc.dram_tensor("w2", [512, 1024], FP8, kind="ExternalInput")  # W2S*w2
    wdq = nc.dram_tensor("wdq", [1024, QP], FP8, kind="ExternalInput")  # ALPHA*wd
    dq = nc.dram_tensor("dq", [R, QP], FP8, kind="ExternalOutput")  # SCALE_OUT*d

    relu = mybir.ActivationFunctionType.Relu

    with tile.TileContext(nc) as tc:
        with (
            tc.tile_pool(name="const", bufs=1) as const,
            tc.tile_pool(name="hpool", bufs=1) as hpool,
            tc.tile_pool(name="wdpool", bufs=11) as wdpool,
            tc.tile_pool(name="opool", bufs=3) as opool,
            tc.tile_pool(name="psA", bufs=2, space="PSUM") as psA,
            tc.tile_pool(name="psB", bufs=3, space="PSUM") as psB,
        ):
            # ---- input loads: blob + wd stream on the sync ring (FIFO:
            # small tensors first), w2 concurrently on the scalar ring ----
            scratch = const.tile([1, 2], F32, name="scratch")
            nc.vector.memset(scratch[:], 0.0)
            # dummy activation: hoists ACT_TABLE_LOAD off the critical path
            nc.scalar.activation(
                scratch[:, 1:2],
                scratch[:, 0:1],
                mybir.ActivationFunctionType.Relu,
                bias=scratch[:, 0:1],
            )
            bt = const.tile([128, BC], BF16, name="bt")
            nc.sync.dma_start(bt[:, 0 : R + 270], blob[:, 0 : R + 270])
            nc.sync.dma_start(bt[:, R + 270 : BC], blob[:, R + 270 : BC])
            w2t = const.tile([128, 4, 1024], FP8, name="w2t")
            nc.sync.dma_start(w2t[:], w2.rearrange("(k p) n -> p k n", p=128))

            xt = bt[0:COND, 0:R]
            w0t = bt[0:COND, R : R + 256]

            def w1s(k, m):
                c0 = R + 270 + k * 512 + m * 128
                return bt[:, c0 : c0 + 128]

            def bias(j):
                return bt[:, R + 256 + j : R + 257 + j]

            # ---- small MLP, transposed layout: h[dout partitions, R free] ----
            h1 = [hpool.tile([128, R], BF16, name=f"h1_{m}") for m in range(2)]
            for m in range(2):
                pt = psA.tile([128, R], F32, name="psmall")
                nc.tensor.matmul(
                    pt[:], w0t[:, m * 128 : (m + 1) * 128], xt, start=True, stop=True
                )
                nc.scalar.activation(h1[m][:], pt[:], relu, bias=bias(m))

            # h2 in fp8 [128, ko=4, R], holds GAMMA*h2
            h2t = hpool.tile([128, 4, R], FP8, name="h2t")
            for m in range(4):
                pt = psA.tile([128, R], F32, name="psmall")
                for k in range(2):
                    nc.tensor.matmul(
                        pt[:],
                        w1s(k, m),
                        h1[k][:],
                        start=(k == 0),
                        stop=(k == 1),
                    )
                nc.scalar.activation(
                    h2t[:, m, :], pt[:], relu, bias=bias(2 + m), scale=GAMMA
                )

            # h3 in fp8 [128, ko=8, R], holds BETA*h3; DoubleRow over 512
            h3t = hpool.tile([128, 8, R], FP8, name="h3t")
            for m in range(8):
                pt = psA.tile([128, R], F32, name="psmall")
                for k in range(2):
                    nc.tensor.matmul(
                        pt[:],
                        w2t[:, 2 * k : 2 * k + 2, m * 128 : (m + 1) * 128],
                        h2t[:, 2 * k : 2 * k + 2, :],
                        start=(k == 0),
                        stop=(k == 1),
                        perf_mode=DR,
                    )
                # psum = W2S*GAMMA*(h2@w2); want BETA*relu(h2@w2 + b2)
                nc.scalar.activation(
                    h3t[:, m, :],
                    pt[:],
                    relu,
                    bias=bias(6 + m),
                    scale=BETA / (W2S * GAMMA),
                )

            # ---- big layer: dq[r, c] = SCALE_OUT * h3.T @ wd, DoubleRow fp8 ----
            # staged units: small first chunks so the first matmuls start early;
            # WID maps each unit to an output-window DMA (last two windows
            # are 1024 wide so the final completion lands earlier)
            UNITS = [256, 256, 512] + [1024] * 6 + [512, 512]
            WID = [0, 0, 0, 0, 1, 1, 2, 2, 3, 4, 5]
            WSTART = {0: 0, 1: 2048, 2: 4096, 3: 6144, 4: 7168, 5: 7680}
            WEND = {0: 2048, 1: 4096, 2: 6144, 3: 7168, 4: 7680, 5: 8192}
            wdq_t = wdq.rearrange("(ko p) n -> p ko n", p=128)  # [128, 8, QP]
            dq_t = dq.rearrange("(s p) c -> p s c", p=128)  # [128, nslots, QP]
            OSC = SCALE_OUT / SCALE  # psum -> out rescale (exact power of 2)
            c0 = 0
            ot = None
            for ui, C in enumerate(UNITS):
                wdt = wdpool.tile([128, 8, C], FP8, name="wdt")
                nc.sync.dma_start(wdt[:], wdq_t[:, :, c0 : c0 + C])
                w = WID[ui]
                wbase, wend = WSTART[w], WEND[w]
                if ot is None:
                    ot = opool.tile([128, nslots, wend - wbase], FP8, name="ot")
                for slot in range(nslots):
                    pt = psB.tile([128, C], F32, name="pbig")
                    for j in range(4):
                        for sub in range(max(1, C // 512)):
                            ssl = slice(sub * 512, min((sub + 1) * 512, C))
                            nc.tensor.matmul(
                                pt[:, ssl],
                                h3t[:, 2 * j : 2 * j + 2, slot * 128 : (slot + 1) * 128],
                                wdt[:, 2 * j : 2 * j + 2, ssl],
                                start=(j == 0),
                                stop=(j == 3),
                                perf_mode=DR,
                            )
                    osl = slice(c0 - wbase, c0 - wbase + C)
                    if slot % 2 == 0:
                        nc.scalar.mul(ot[:, slot, osl], pt[:], OSC)
                    else:
                        nc.vector.tensor_scalar_mul(ot[:, slot, osl], pt[:], OSC)
                c0 += C
                if c0 == wend:
                    nc.scalar.dma_start(dq_t[:, :, wbase:wend], ot[:])
                    ot = None
    nc.compile()
    return nc


def _ensure_ntff_hook():
    """Provide antenv.axon_hooks (absent in this image) so trace=True works."""
    import sys
    import types

    try:
        from antenv.axon_hooks import get_axon_ntff_profile_hook  # noqa: F401

        return
    except ImportError:
        pass
    try:
        import antenv
        from trn_agent_boot.trn_boot import _ntff_profile_via_ctypes

        hook = _ntff_profile_via_ctypes("/opt/axon/libaxon_pjrt.so")
        mod = types.ModuleType("antenv.axon_hooks")
        mod._hook = hook
        mod.set_axon_ntff_profile_hook = lambda h: setattr(mod, "_hook", h)
        mod.get_axon_ntff_profile_hook = lambda: mod._hook
        sys.modules["antenv.axon_hooks"] = mod
        antenv.axon_hooks = mod
    except Exception:
        pass


def _exact_h3(x, ws, bs):
    h = x.astype(np.float64)
    for i in range(3):
        h = np.maximum(h @ ws[i].astype(np.float64) + bs[i].astype(np.float64), 0)
    return h


def kernel(**inputs) -> np.ndarray:
    global last_results
    x = np.ascontiguousarray(inputs["x"], dtype=np.float32)
    gumbel = np.ascontiguousarray(inputs["gumbel"], dtype=np.float32)
    bw = [np.asarray(inputs[f"bw{i}"], dtype=np.float32) for i in range(4)]
    bb = [np.asarray(inputs[f"bb{i}"], dtype=np.float32) for i in range(4)]
    sw = [np.asarray(inputs[f"sw{i}"], dtype=np.float32) for i in range(4)]
    sb = [np.asarray(inputs[f"sb{i}"], dtype=np.float32) for i in range(4)]

    mask_big = x[:, 0] > 0.0
    b = int(mask_big.sum())
    # stable sort: big rows first, original order within groups
    perm = np.argsort(~mask_big, kind="stable")
    x_sorted = x[perm]

    def wd_of(w3):
        wd = w3[:, 0::2] - w3[:, 1::2]
        # pad each 8160-col quarter independently to 8192 cols
        wdp = np.zeros((1024, QP * 4), dtype=np.float32)
        for q in range(4):
            wdp[:, q * QP : q * QP + QCOLS] = wd[:, q * QCOLS : (q + 1) * QCOLS]
        wdp *= ALPHA
        np.clip(wdp, -240.0, 240.0, out=wdp)
        return wdp.astype(NP_FP8)

    wd8 = {"big": wd_of(bw[3]), "small": wd_of(sw[3])}
    wd_f32 = {
        "big": bw[3][:, 0::2] - bw[3][:, 1::2],
        "small": sw[3][:, 0::2] - sw[3][:, 1::2],
    }
    bdd = {"big": bb[3][0::2] - bb[3][1::2], "small": sb[3][0::2] - sb[3][1::2]}
    small_w = {"big": bw[:3], "small": sw[:3]}
    small_b = {"big": bb[:3], "small": sb[:3]}

    # chunk -> expert assignment over sorted rows
    bigchunks = [c for c in range(4) if 128 * c < b]
    smallchunks = [c for c in range(4) if 128 * (c + 1) > b]
    if b == 0:
        groups = [("small", [0, 1]), ("small", [2, 3])]
    elif b == B:
        groups = [("big", [0, 1]), ("big", [2, 3])]
    else:
        groups = [("big", bigchunks), ("small", smallchunks)]
    nslots = max(len(g[1]) for g in groups)
    slots = []
    for exp, chunks in groups:
        padded = list(chunks) + [chunks[-1]] * (nslots - len(chunks))
        slots.append((exp, padded))

    if nslots not in _program_cache:
        _program_cache[nslots] = build_program(nslots)
    nc = _program_cache[nslots]

    R = nslots * 128

    def blob_pack(xT_g, ws, bs):
        blob = np.zeros((128, blob_cols(R)), dtype=NP_BF16)
        blob[:COND, 0:R] = xT_g.astype(NP_BF16)
        blob[:COND, R : R + 256] = ws[0].astype(NP_BF16)
        bcols = np.empty((128, 14), dtype=np.float32)
        bcols[:, 0:2] = bs[0].reshape(2, 128).T
        bcols[:, 2:6] = (GAMMA * bs[1]).reshape(4, 128).T
        bcols[:, 6:14] = (BETA * bs[2]).reshape(8, 128).T
        blob[:, R + 256 : R + 270] = bcols.astype(NP_BF16)
        blob[:, R + 270 : R + 1294] = (
            ws[1].reshape(2, 128, 512).transpose(1, 0, 2).reshape(128, 1024)
        ).astype(NP_BF16)
        return blob

    in_maps = []
    for g, (exp, chunks) in enumerate(slots):
        rows = np.concatenate([np.arange(128 * c, 128 * (c + 1)) for c in chunks])
        xT_g = np.ascontiguousarray(x_sorted[rows].T)
        blob = blob_pack(xT_g, small_w[exp], small_b[exp])
        w2f = np.clip(small_w[exp][2] * W2S, -240, 240).astype(NP_FP8)
        for q in range(4):
            qsl = slice(q * QP, (q + 1) * QP)
            in_maps.append(
                {
                    "blob": blob,
                    "w2": w2f,
                    "wdq": np.ascontiguousarray(wd8[exp][:, qsl]),
                }
            )

    trace = bool(int(os.environ.get("CC_KERNEL_TRACE", "0")))
    if trace:
        _ensure_ntff_hook()
    try:
        res = run_bass_kernel_spmd(
            nc,
            in_maps,
            core_ids=list(range(NCORES)),
            trace=trace,
            trace_cores=list(range(NCORES)) if trace else None,
        )
    except Exception:
        if not trace:
            raise
        res = run_bass_kernel_spmd(nc, in_maps, core_ids=list(range(NCORES)))
    last_results = res

    # ---- assemble d (unscaled) in sorted row order ----
    d_sorted = np.empty((B, E), dtype=np.float32)
    for g, (exp, chunks) in enumerate(slots):
        isbig = exp == "big"
        for s, c in enumerate(chunks):
            r0, r1 = 128 * c, 128 * (c + 1)
            if 0 < b < B:
                sel = (np.arange(r0, r1) < b) == isbig
            else:
                sel = np.ones(128, dtype=bool)
            if not sel.any():
                continue
            for q in range(4):
                shard = res.results[g * 4 + q]["dq"]
                d_sorted[r0:r1, q * QCOLS : (q + 1) * QCOLS][sel] = (
                    shard[s * 128 : (s + 1) * 128, :QCOLS][sel].astype(np.float32)
                    / SCALE_OUT
                )

    # unsort rows
    d_full = np.empty_like(d_sorted)
    d_full[perm] = d_sorted
    global last_d_full
    last_d_full = d_full

    # exact gd and per-row bdd; margins
    bdd_sel = np.where(mask_big[:, None], bdd["big"][None, :], bdd["small"][None, :])
    gd = gumbel[:, :, 1].astype(np.float32) - gumbel[:, :, 0].astype(np.float32)
    t_full = d_full + bdd_sel - gd
    dec_full = t_full >= 0.0

    # ---- exact patch of near-tie edges ----
    thr = TOL_ABS + TOL_REL * np.abs(d_full)
    near_r, near_c = np.nonzero(np.abs(t_full) < thr)
    if near_r.size:
        gde = (
            gumbel[near_r, near_c, 1].astype(np.float64)
            - gumbel[near_r, near_c, 0].astype(np.float64)
        )
        for exp, msk in (("big", mask_big), ("small", ~mask_big)):
            selp = msk[near_r]
            if not selp.any():
                continue
            r, c = near_r[selp], near_c[selp]
            ws = small_w[exp]
            bs = small_b[exp]
            h3e = _exact_h3(x, ws, bs)  # [B, 1024] float64
            d = np.einsum("ij,ji->i", h3e[r], wd_f32[exp][:, c].astype(np.float64))
            m = d + bdd[exp][c] - gde[selp]
            dec_full[r, c] = m >= 0
    dec_full = dec_full.astype(np.float32)

    # ---- scatter to symmetric adjacency ----
    iu, ju = np.triu_indices(N_NODES, k=1)
    flat_idx = iu * N_NODES + ju
    out = np.zeros((B, N_NODES * N_NODES), dtype=np.float32)
    out[:, flat_idx] = dec_full
    out = out.reshape(B, N_NODES, N_NODES)
    out = out + np.swapaxes(out, 1, 2)
    return out


# revision 37
# speedup vs baseline: 1.1106x; 1.1106x over previous
"""Trainium2 Bass kernel for nn_MixMLP (moe_routing) — fp8 DoubleRow, v4.

Strategy:
  - Output is binary: y_hard + y_soft - stop_grad(y_soft) == y_hard numerically,
    so each edge decision is  (logit0 - logit1) + (gum0 - gum1) >= 0.
  - Only the DIFFERENCE of adjacent final-layer columns matters:
        d = h3 @ wd,  wd = w3[:, 0::2] - w3[:, 1::2]   (1024 x 32640)
    decision = (d + bdd - gd) >= 0,  gd = gum1 - gum0, bdd = b3[0::2]-b3[1::2].
  - Rows are routed to one expert by mask = x[:,0] > 0. Host sorts rows so
    row-chunks of 128 are single-expert; 8 cores = 2 row-groups (one per
    expert) x 4 column-quarters of wd.
  - Device computes ONLY d (scaled). MLP layers 1-2 in bf16; h2 emitted as
    fp8 so the h3 layer and the big layer both run DoubleRow fp8
    (256-contraction, 2x rate). Output d in fp8 (x SCALE_OUT). gumbel never
    touches the device.
  - Perf structure: one coalesced bf16 input blob + one fp8 w2 + 8x1MB wd
    stream, all on the sync HWDGE ring in FIFO order (small tensors first);
    2-bank PSUM tiles evacuated by single [128,1024] copies alternating
    ACT/DVE; output DMAs (4 x 0.5MB fp8) on the scalar HWDGE ring.
  - Host: dec = (t >= 0), t = d + bdd - gd; near-ties |t| < 7e-3+0.05|d|
    recomputed exactly in float64, then scattered into the symmetric
    adjacency. Result is exact regardless of device matmul precision.
"""

import os
import numpy as np
import ml_dtypes

import concourse.bass as bass
import concourse.mybir as mybir
import concourse.tile as tile
from concourse import bacc
from concourse.bass_utils import run_bass_kernel_spmd

B = 512
COND = 64
N_NODES = 256
E = 32640  # upper-tri edges
NCORES = 8
QCOLS = E // 4  # 8160 columns of wd per core
QP = 8192  # padded to multiple of 1024
ARCH = [256, 512, 1024]

ALPHA = 512.0  # wd scale into fp8
BETA = 32.0  # h3 scale into fp8
GAMMA = 32.0  # h2 scale into fp8
W2S = 16.0  # w2 scale into fp8
SCALE = ALPHA * BETA  # big-layer psum holds SCALE*d
SCALE_OUT = 512.0  # output tensor holds SCALE_OUT*d

F32 = mybir.dt.float32
BF16 = mybir.dt.bfloat16
FP8 = mybir.dt.float8e4
NP_FP8 = ml_dtypes.float8_e4m3
NP_BF16 = ml_dtypes.bfloat16

# |t| < TOL_ABS + TOL_REL*|d| edges are recomputed exactly on host
TOL_ABS = 7.0e-3
TOL_REL = 0.05  # covers fp8 output quantization of d

# input blob column layout (bf16, 128 partitions), R = nslots*128:
#   [0:R)           xT   (64 partitions used)
#   [R:R+256)       w0   (64 partitions used)
#   [R+256:R+270)   packed biases: b0 x2 | GAMMA*b1 x4 | BETA*b2 x8
#   [R+270:R+1294)  w1   as k-major [k=0 512 cols | k=1 512 cols]
# it is DMA'd in two pieces: [0:R+270) first (unblocks h1), then w1
def blob_cols(R):
    return R + 1294

_program_cache = {}
last_results = None  # BassKernelResults of the most recent device run


def build_program(nslots: int):
    """One SPMD program: R = nslots*128 rows, one expert, one wd quarter."""
    R = nslots * 128
    DR = mybir.MatmulPerfMode.DoubleRow
    nc = bacc.Bacc(None, target_bir_lowering=False)

    BC = blob_cols(R)
    blob = nc.dram_tensor("blob", [128, BC], BF16, kind="ExternalInput")
    w2 = nc.dram_tensor("w2", [512, 1024], FP8, kind="ExternalInput")  # W2S*w2
    wdq = nc.dram_tensor("wdq", [1024, QP], FP8, kind="ExternalInput")  # ALPHA*wd
    dq = nc.dram_tensor("dq", [R, QP], FP8, kind="ExternalOutput")  # SCALE_OUT*d

    relu = mybir.ActivationFunctionType.Relu

    with tile.TileContext(nc) as tc:
        with (
            tc.tile_pool(name="const", bufs=1) as const,
            tc.tile_pool(name="hpool", bufs=1) as hpool,
            tc.tile_pool(name="wdpool", bufs=9) as wdpool,
            tc.tile_pool(name="opool", bufs=3) as opool,
            tc.tile_pool(name="psA", bufs=2, space="PSUM") as psA,
            tc.tile_pool(name="psB", bufs=3, space="PSUM") as psB,
        ):
            # ---- input loads: blob + wd stream on the sync ring (FIFO:
            # small tensors first), w2 concurrently on the scalar ring ----
            scratch = const.tile([1, 2], F32, name="scratch")
            nc.vector.memset(scratch[:], 0.0)
            # dummy activation: hoists ACT_TABLE_LOAD off the critical path
            nc.scalar.activation(
                scratch[:, 1:2],
                scratch[:, 0:1],
                mybir.ActivationFunctionType.Relu,
                bias=scratch[:, 0:1],
            )
            bt = const.tile([128, BC], BF16, name="bt")
            nc.sync.dma_start(bt[:, 0 : R + 270], blob[:, 0 : R + 270])
            nc.sync.dma_start(bt[:, R + 270 : BC], blob[:, R + 270 : BC])
            w2t = const.tile([128, 4, 1024], FP8, name="w2t")
            nc.sync.dma_start(w2t[:], w2.rearrange("(k p) n -> p k n", p=128))

            xt = bt[0:COND, 0:R]
            w0t = bt[0:COND, R : R + 256]

            def w1s(k, m):
                c0 = R + 270 + k * 512 + m * 128
                return bt[:, c0 : c0 + 128]

            def bias(j):
                return bt[:, R + 256 + j : R + 257 + j]

            # ---- small MLP, transposed layout: h[dout partitions, R free] ----
            h1 = [hpool.tile([128, R], BF16, name=f"h1_{m}") for m in range(2)]
            for m in range(2):
                pt = psA.tile([128, R], F32, name="psmall")
                nc.tensor.matmul(
                    pt[:], w0t[:, m * 128 : (m + 1) * 128], xt, start=True, stop=True
                )
                nc.scalar.activation(h1[m][:], pt[:], relu, bias=bias(m))

            # h2 in fp8 [128, ko=4, R], holds GAMMA*h2
            h2t = hpool.tile([128, 4, R], FP8, name="h2t")
            for m in range(4):
                pt = psA.tile([128, R], F32, name="psmall")
                for k in range(2):
                    nc.tensor.matmul(
                        pt[:],
                        w1s(k, m),
                        h1[k][:],
                        start=(k == 0),
                        stop=(k == 1),
                    )
                nc.scalar.activation(
                    h2t[:, m, :], pt[:], relu, bias=bias(2 + m), scale=GAMMA
                )

            # h3 in fp8 [128, ko=8, R], holds BETA*h3; DoubleRow over 512
            h3t = hpool.tile([128, 8, R], FP8, name="h3t")
            for m in range(8):
                pt = psA.tile([128, R], F32, name="psmall")
                for k in range(2):
                    nc.tensor.matmul(
                        pt[:],
                        w2t[:, 2 * k : 2 * k + 2, m * 128 : (m + 1) * 128],
                        h2t[:, 2 * k : 2 * k + 2, :],
                        start=(k == 0),
                        stop=(k == 1),
                        perf_mode=DR,
                    )
                # psum = W2S*GAMMA*(h2@w2); want BETA*relu(h2@w2 + b2)
                nc.scalar.activation(
                    h3t[:, m, :],
                    pt[:],
                    relu,
                    bias=bias(6 + m),
                    scale=BETA / (W2S * GAMMA),
                )

            # ---- big layer: dq[r, c] = SCALE_OUT * h3.T @ wd, DoubleRow fp8 ----
            # staged units: small first chunks so the first matmuls start early;
            # WID maps each unit to an output-window DMA (last two windows
            # are 1024 wide so the final completion lands earlier)
            UNITS = [256, 256, 512] + [1024] * 7
            WID = [0, 0, 0, 0, 1, 1, 2, 2, 3, 4]
            WSTART = {0: 0, 1: 2048, 2: 4096, 3: 6144, 4: 7168}
            WEND = {0: 2048, 1: 4096, 2: 6144, 3: 7168, 4: 8192}
            wdq_t = wdq.rearrange("(ko p) n -> p ko n", p=128)  # [128, 8, QP]
            dq_t = dq.rearrange("(s p) c -> p s c", p=128)  # [128, nslots, QP]
            OSC = SCALE_OUT / SCALE  # psum -> out rescale (exact power of 2)
            c0 = 0
            ot = None
            for ui, C in enumerate(UNITS):
                wdt = wdpool.tile([128, 8, C], FP8, name="wdt")
                nc.sync.dma_start(wdt[:], wdq_t[:, :, c0 : c0 + C])
                w = WID[ui]
                wbase, wend = WSTART[w], WEND[w]
                if ot is None:
                    ot = opool.tile([128, nslots, wend - wbase], FP8, name="ot")
                for slot in range(nslots):
                    pt = psB.tile([128, C], F32, name="pbig")
                    for j in range(4):
                        for sub in range(max(1, C // 512)):
                            ssl = slice(sub * 512, min((sub + 1) * 512, C))
                            nc.tensor.matmul(
                                pt[:, ssl],
                                h3t[:, 2 * j : 2 * j + 2, slot * 128 : (slot + 1) * 128],
                                wdt[:, 2 * j : 2 * j + 2, ssl],
                                start=(j == 0),
                                stop=(j == 3),
                                perf_mode=DR,
                            )
                    osl = slice(c0 - wbase, c0 - wbase + C)
                    if slot % 2 == 0:
                        nc.scalar.mul(ot[:, slot, osl], pt[:], OSC)
                    else:
                        nc.vector.tensor_scalar_mul(ot[:, slot, osl], pt[:], OSC)
                c0 += C
                if c0 == wend:
                    nc.scalar.dma_start(dq_t[:, :, wbase:wend], ot[:])
                    ot = None
    nc.compile()
    return nc


def _ensure_ntff_hook():
    """Provide antenv.axon_hooks (absent in this image) so trace=True works."""
    import sys
    import types

    try:
        from antenv.axon_hooks import get_axon_ntff_profile_hook  # noqa: F401

        return
    except ImportError:
        pass
    try:
        import antenv
        from trn_agent_boot.trn_boot import _ntff_profile_via_ctypes

        hook = _ntff_profile_via_ctypes("/opt/axon/libaxon_pjrt.so")
        mod = types.ModuleType("antenv.axon_hooks")
        mod._hook = hook
        mod.set_axon_ntff_profile_hook = lambda h: setattr(mod, "_hook", h)
        mod.get_axon_ntff_profile_hook = lambda: mod._hook
        sys.modules["antenv.axon_hooks"] = mod
        antenv.axon_hooks = mod
    except Exception:
        pass


def _exact_h3(x, ws, bs):
    h = x.astype(np.float64)
    for i in range(3):
        h = np.maximum(h @ ws[i].astype(np.float64) + bs[i].astype(np.float64), 0)
    return h


def kernel(**inputs) -> np.ndarray:
    global last_results
    x = np.ascontiguousarray(inputs["x"], dtype=np.float32)
    gumbel = np.ascontiguousarray(inputs["gumbel"], dtype=np.float32)
    bw = [np.asarray(inputs[f"bw{i}"], dtype=np.float32) for i in range(4)]
    bb = [np.asarray(inputs[f"bb{i}"], dtype=np.float32) for i in range(4)]
    sw = [np.asarray(inputs[f"sw{i}"], dtype=np.float32) for i in range(4)]
    sb = [np.asarray(inputs[f"sb{i}"], dtype=np.float32) for i in range(4)]

    mask_big = x[:, 0] > 0.0
    b = int(mask_big.sum())
    # stable sort: big rows first, original order within groups
    perm = np.argsort(~mask_big, kind="stable")
    x_sorted = x[perm]

    def wd_of(w3):
        wd = w3[:, 0::2] - w3[:, 1::2]
        # pad each 8160-col quarter independently to 8192 cols
        wdp = np.zeros((1024, QP * 4), dtype=np.float32)
        for q in range(4):
            wdp[:, q * QP : q * QP + QCOLS] = wd[:, q * QCOLS : (q + 1) * QCOLS]
        wdp *= ALPHA
        np.clip(wdp, -240.0, 240.0, out=wdp)
        return wdp.astype(NP_FP8)

    wd8 = {"big": wd_of(bw[3]), "small": wd_of(sw[3])}
    wd_f32 = {
        "big": bw[3][:, 0::2] - bw[3][:, 1::2],
        "small": sw[3][:, 0::2] - sw[3][:, 1::2],
    }
    bdd = {"big": bb[3][0::2] - bb[3][1::2], "small": sb[3][0::2] - sb[3][1::2]}
    small_w = {"big": bw[:3], "small": sw[:3]}
    small_b = {"big": bb[:3], "small": sb[:3]}

    # chunk -> expert assignment over sorted rows
    bigchunks = [c for c in range(4) if 128 * c < b]
    smallchunks = [c for c in range(4) if 128 * (c + 1) > b]
    if b == 0:
        groups = [("small", [0, 1]), ("small", [2, 3])]
    elif b == B:
        groups = [("big", [0, 1]), ("big", [2, 3])]
    else:
        groups = [("big", bigchunks), ("small", smallchunks)]
    nslots = max(len(g[1]) for g in groups)
    slots = []
    for exp, chunks in groups:
        padded = list(chunks) + [chunks[-1]] * (nslots - len(chunks))
        slots.append((exp, padded))

    if nslots not in _program_cache:
        _program_cache[nslots] = build_program(nslots)
    nc = _program_cache[nslots]

    R = nslots * 128

    def blob_pack(xT_g, ws, bs):
        blob = np.zeros((128, blob_cols(R)), dtype=NP_BF16)
        blob[:COND, 0:R] = xT_g.astype(NP_BF16)
        blob[:COND, R : R + 256] = ws[0].astype(NP_BF16)
        bcols = np.empty((128, 14), dtype=np.float32)
        bcols[:, 0:2] = bs[0].reshape(2, 128).T
        bcols[:, 2:6] = (GAMMA * bs[1]).reshape(4, 128).T
        bcols[:, 6:14] = (BETA * bs[2]).reshape(8, 128).T
        blob[:, R + 256 : R + 270] = bcols.astype(NP_BF16)
        blob[:, R + 270 : R + 1294] = (
            ws[1].reshape(2, 128, 512).transpose(1, 0, 2).reshape(128, 1024)
        ).astype(NP_BF16)
        return blob

    in_maps = []
    for g, (exp, chunks) in enumerate(slots):
        rows = np.concatenate([np.arange(128 * c, 128 * (c + 1)) for c in chunks])
        xT_g = np.ascontiguousarray(x_sorted[rows].T)
        blob = blob_pack(xT_g, small_w[exp], small_b[exp])
        w2f = np.clip(small_w[exp][2] * W2S, -240, 240).astype(NP_FP8)
        for q in range(4):
            qsl = slice(q * QP, (q + 1) * QP)
            in_maps.append(
                {
                    "blob": blob,
                    "w2": w2f,
                    "wdq": np.ascontiguousarray(wd8[exp][:, qsl]),
                }
            )

    trace = bool(int(os.environ.get("CC_KERNEL_TRACE", "0")))
    if trace:
        _ensure_ntff_hook()
    try:
        res = run_bass_kernel_spmd(
            nc,
            in_maps,
            core_ids=list(range(NCORES)),
            trace=trace,
            trace_cores=list(range(NCORES)) if trace else None,
        )
    except Exception:
        if not trace:
            raise
        res = run_bass_kernel_spmd(nc, in_maps, core_ids=list(range(NCORES)))
    last_results = res

    # ---- assemble d (unscaled) in sorted row order ----
    d_sorted = np.empty((B, E), dtype=np.float32)
    for g, (exp, chunks) in enumerate(slots):
        isbig = exp == "big"
        for s, c in enumerate(chunks):
            r0, r1 = 128 * c, 128 * (c + 1)
            if 0 < b < B:
                sel = (np.arange(r0, r1) < b) == isbig
            else:
                sel = np.ones(128, dtype=bool)
            if not sel.any():
                continue
            for q in range(4):
                shard = res.results[g * 4 + q]["dq"]
                d_sorted[r0:r1, q * QCOLS : (q + 1) * QCOLS][sel] = (
                    shard[s * 128 : (s + 1) * 128, :QCOLS][sel].astype(np.float32)
                    / SCALE_OUT
                )

    # unsort rows
    d_full = np.empty_like(d_sorted)
    d_full[perm] = d_sorted
    global last_d_full
    last_d_full = d_full

    # exact gd and per-row bdd; margins
    bdd_sel = np.where(mask_big[:, None], bdd["big"][None, :], bdd["small"][None, :])
    gd = gumbel[:, :, 1].astype(np.float32) - gumbel[:, :, 0].astype(np.float32)
    t_full = d_full + bdd_sel - gd
    dec_full = t_full >= 0.0

    # ---- exact patch of near-tie edges ----
    thr = TOL_ABS + TOL_REL * np.abs(d_full)
    near_r, near_c = np.nonzero(np.abs(t_full) < thr)
    if near_r.size:
        gde = (
            gumbel[near_r, near_c, 1].astype(np.float64)
            - gumbel[near_r, near_c, 0].astype(np.float64)
        )
        for exp, msk in (("big", mask_big), ("small", ~mask_big)):
            selp = msk[near_r]
            if not selp.any():
                continue
            r, c = near_r[selp], near_c[selp]
            ws = small_w[exp]
            bs = small_b[exp]
            h3e = _exact_h3(x, ws, bs)  # [B, 1024] float64
            d = np.einsum("ij,ji->i", h3e[r], wd_f32[exp][:, c].astype(np.float64))
            m = d + bdd[exp][c] - gde[selp]
            dec_full[r, c] = m >= 0
    dec_full = dec_full.astype(np.float32)

    # ---- scatter to symmetric adjacency ----
    iu, ju = np.triu_indices(N_NODES, k=1)
    flat_idx = iu * N_NODES + ju
    out = np.zeros((B, N_NODES * N_NODES), dtype=np.float32)
    out[:, flat_idx] = dec_full
    out = out.reshape(B, N_NODES, N_NODES)
    out = out + np.swapaxes(out, 1, 2)
    return out
